# revision 1
# baseline (speedup 1.0000x reference)
"""Trainium2 Bass kernel for a selective-SSM block (LN -> x_proj ->
softplus(dt_proj) -> diagonal SSM scan over L -> out_proj).

Sharding: 8 cores = 2 batches x 4 D-quarters. Each core runs the scan for its
512 channels over the full sequence (channels on partitions, time on the free
dim, one tensor_tensor_scan per SSM state index n). out_proj partials (each
core contracts over its own 512 channels) are summed on the host.

Layout trick: the host passes x[b].T with rows permuted so each core's own
D-quarter occupies chunk rows 0..511; all d-contractions (LN stats, x_proj)
are permutation-invariant because the matching weight rows are permuted too.
"""

import os
import sys

import numpy as np

try:
    import concourse.bass as bass
except ImportError:
    sys.path.insert(0, "/opt/trn_rl_repo")
    import concourse.bass as bass

import concourse.tile as tile
from concourse import mybir
from concourse.bass_utils import run_bass_kernel_spmd

F32 = mybir.dt.float32
F32R = mybir.dt.float32r
BF16 = mybir.dt.bfloat16
AF = mybir.ActivationFunctionType
ALU = mybir.AluOpType

B, L, D, N, R = 2, 4096, 2048, 64, 128
P_PROJ = 2 * N + R  # 256
DQ = D // 4  # channels per core: 512
NCH = DQ // 128  # own d-chunks: 4
NCHALL = D // 128  # all d-chunks: 16
EPS = 1e-5

LAST_RESULTS = None  # BassKernelResults of the most recent run (for test.py)

_PROGRAM_CACHE = {}


def _r(ap):
    return ap.bitcast(F32R)


def _build(nc, L_):
    LH = L_ // 2  # half length
    TB = min(512, LH)  # phase A/C time block
    NTB = LH // TB
    TS = min(2048, LH)  # scan time block
    NS = LH // TS

    xT = nc.dram_tensor("xT", [D, L_], F32R, kind="ExternalInput")
    wxF = nc.dram_tensor("wxF", [D, P_PROJ], F32R, kind="ExternalInput")
    g0c = nc.dram_tensor("g0c", [128, 2], F32, kind="ExternalInput")
    c0c = nc.dram_tensor("c0c", [128, 2], F32, kind="ExternalInput")
    dtwT = nc.dram_tensor("dtwT", [R, DQ], F32R, kind="ExternalInput")
    dtbc = nc.dram_tensor("dtbc", [128, NCH], F32, kind="ExternalInput")
    acols = nc.dram_tensor("acols", [128, NCH, N], F32, kind="ExternalInput")
    wbc = nc.dram_tensor("wbc", [128, NCH], F32, kind="ExternalInput")
    bbc = nc.dram_tensor("bbc", [128, NCH], F32, kind="ExternalInput")
    dpc = nc.dram_tensor("dpc", [128, NCH], F32, kind="ExternalInput")
    woT = nc.dram_tensor("woT", [DQ, D], F32R, kind="ExternalInput")
    idc = nc.dram_tensor("idc", [128, N], F32R, kind="ExternalInput")
    onesc = nc.dram_tensor("onesc", [128, 128], F32R, kind="ExternalInput")
    out_part = nc.dram_tensor("out_part", [D, L_], F32, kind="ExternalOutput")

    with tile.TileContext(nc) as tc:
        with (
            tc.tile_pool(name="single", bufs=1) as single,
            tc.tile_pool(name="persist", bufs=1) as persist,
            tc.tile_pool(name="xin", bufs=2) as xin,
            tc.tile_pool(name="xown", bufs=5) as xown,
            tc.tile_pool(name="wrk", bufs=2) as wrk,
            tc.tile_pool(name="stats", bufs=1) as stats,
            tc.tile_pool(name="stats2", bufs=1) as stats2,
            tc.tile_pool(name="wpool", bufs=3) as wpool,
            tc.tile_pool(name="scan", bufs=2) as scanp,
            tc.tile_pool(name="hpool", bufs=2) as hpool,
            tc.tile_pool(name="zpool", bufs=1) as zpool,
            tc.tile_pool(name="cpool", bufs=2) as cpool,
            tc.tile_pool(name="psum", bufs=1, space=bass.MemorySpace.PSUM) as psum,
        ):
            # --- constants ---
            ones128 = single.tile([128, 128], F32R)
            nc.sync.dma_start(ones128, onesc[:, :])
            id_sb = single.tile([128, N], F32R)
            nc.sync.dma_start(id_sb, idc[:, :])
            eps_sb = single.tile([128, 1], F32)
            nc.vector.memset(eps_sb, EPS)
            g0_sb = single.tile([128, 2], F32)
            nc.sync.dma_start(g0_sb, g0c[:, :])
            c0_sb = single.tile([128, 2], F32)
            nc.sync.dma_start(c0_sb, c0c[:, :])
            dtb_sb = single.tile([128, NCH], F32)
            nc.sync.dma_start(dtb_sb, dtbc[:, :])
            a_sb = single.tile([128, NCH, N], F32)
            nc.sync.dma_start(a_sb, acols[:, :, :])
            w_sb = single.tile([128, NCH], F32)
            nc.sync.dma_start(w_sb, wbc[:, :])
            b_sb = single.tile([128, NCH], F32)
            nc.sync.dma_start(b_sb, bbc[:, :])
            dp_sb = single.tile([128, NCH], F32)
            nc.sync.dma_start(dp_sb, dpc[:, :])
            dtw_sb = single.tile([128, NCH, 128], F32R)
            for c in range(NCH):
                nc.sync.dma_start(dtw_sb[:, c, :], dtwT[:, c * 128 : (c + 1) * 128])

            # persistent per-half buffers
            delta_sb = persist.tile([128, NCH, LH], F32, tag="delta")
            dxn_sb = persist.tile([128, NCH, LH], BF16, tag="dxn")
            bc_sb = persist.tile([128, LH], F32R, tag="bc")  # rows 0-63 B, 64-127 C
            y_sb = persist.tile([128, NCH, LH], F32, tag="y")
            hcarry = persist.tile([128, NCH * N], F32, tag="hcarry")

            for half in range(2):
                t0h = half * LH
                # ---------------- phase A ----------------
                for itb in range(NTB):
                    t0 = t0h + itb * TB
                    tsl = slice(itb * TB, (itb + 1) * TB)
                    ps_sx = psum.tile([128, TB], F32, tag="ps1a")
                    ps_sxx = psum.tile([128, TB], F32, tag="ps1b")
                    ps_g0 = psum.tile([128, TB], F32, tag="ps2a")
                    ps_g1 = psum.tile([128, TB], F32, tag="ps2b")
                    own_tiles = {}
                    for c in range(NCHALL):
                        if c < NCH:  # own-quarter chunk (host row-permutation)
                            xc = xown.tile([128, TB], F32R, tag="xown")
                            own_tiles[c] = xc
                        else:
                            xc = xin.tile([128, TB], F32R, tag="xin")
                        nc.sync.dma_start(xc, xT[c * 128 : (c + 1) * 128, t0 : t0 + TB])
                        x2 = wrk.tile([128, TB], F32R, tag="wa")
                        nc.scalar.square(x2, xc.bitcast(F32))
                        st = c == 0
                        sp = c == NCHALL - 1
                        nc.tensor.matmul(ps_sx, ones128, xc, start=st, stop=sp)
                        nc.tensor.matmul(ps_sxx, ones128, x2, start=st, stop=sp)
                        wx = wpool.tile([128, P_PROJ], F32R, tag="wx")
                        nc.sync.dma_start(wx, wxF[c * 128 : (c + 1) * 128, :])
                        nc.tensor.matmul(ps_g0, wx[:, 0:128], xc, start=st, stop=sp)
                        nc.tensor.matmul(ps_g1, wx[:, 128:256], xc, start=st, stop=sp)

                    # stats: mean, rstd (all partition rows carry the same value)
                    mean_b = stats.tile([128, TB], F32, tag="mean")
                    nc.scalar.mul(mean_b, ps_sx, 1.0 / D)
                    msq = stats.tile([128, TB], F32, tag="sA")
                    nc.scalar.mul(msq, ps_sxx, 1.0 / D)
                    m2 = stats.tile([128, TB], F32, tag="sB")
                    nc.vector.tensor_mul(m2, mean_b, mean_b)
                    nc.vector.tensor_sub(msq, msq, m2)  # msq <- var
                    # rstd = exp(-0.5*ln(var+eps)): stays inside the exp/ln ACT set
                    nc.scalar.activation(m2, msq, AF.Ln, bias=eps_sb[:, 0:1])
                    rstd_b = stats2.tile([128, TB], F32, tag="rstd")
                    nc.scalar.activation(rstd_b, m2, AF.Exp, scale=-0.5)
                    mr_b = stats2.tile([128, TB], F32, tag="mr")
                    nc.vector.tensor_mul(mr_b, mean_b, rstd_b)

                    # proj = rstd*G - (mr*g0 - c0)   (LN folded into x_proj)
                    dr_sb = wrk.tile([128, TB], F32R, tag="drt")
                    for ph, ps_g in enumerate([ps_g0, ps_g1]):
                        s1 = wrk.tile([128, TB], F32, tag="wb")
                        nc.vector.tensor_mul(s1, ps_g, rstd_b)
                        s2 = wrk.tile([128, TB], F32, tag="wc")
                        nc.vector.tensor_scalar(
                            s2,
                            mr_b,
                            g0_sb[:, ph : ph + 1],
                            c0_sb[:, ph : ph + 1],
                            op0=ALU.mult,
                            op1=ALU.subtract,
                        )
                        tgt = dr_sb if ph == 0 else bc_sb[:, tsl]
                        nc.vector.tensor_sub(tgt, s1, s2)

                    # dt_proj + softplus -> delta; xn + delta*xn for own chunks
                    for c in range(NCH):
                        ps_dt = psum.tile([128, TB], F32, tag="ps1c")
                        nc.tensor.matmul(
                            ps_dt, dtw_sb[:, c, :], dr_sb, start=True, stop=True
                        )
                        # softplus(z) = relu(z) + ln(1 + exp(-|z|)); all four
                        # ACT funcs live in the natural_log_exp table set
                        dsl = delta_sb[:, c, tsl]
                        t_abs = wrk.tile([128, TB], F32, tag="wa")
                        nc.scalar.activation(
                            t_abs, ps_dt, AF.Abs, bias=dtb_sb[:, c : c + 1]
                        )
                        nc.scalar.activation(t_abs, t_abs, AF.Exp, scale=-1.0)
                        nc.scalar.activation(t_abs, t_abs, AF.Ln, bias=1.0)
                        t_r = wrk.tile([128, TB], F32, tag="wb")
                        nc.scalar.activation(
                            t_r, ps_dt, AF.Relu, bias=dtb_sb[:, c : c + 1]
                        )
                        nc.vector.tensor_add(dsl, t_abs, t_r)
                        xc = own_tiles[c]
                        t1 = wrk.tile([128, TB], F32, tag="wa")
                        nc.vector.tensor_mul(t1, xc.bitcast(F32), rstd_b)
                        t2 = wrk.tile([128, TB], F32, tag="wb")
                        nc.vector.tensor_sub(t2, t1, mr_b)
                        xnc = wrk.tile([128, TB], F32, tag="wc")
                        nc.scalar.activation(
                            xnc,
                            t2,
                            AF.Identity,
                            bias=b_sb[:, c : c + 1],
                            scale=w_sb[:, c : c + 1],
                        )
                        nc.vector.tensor_mul(dxn_sb[:, c, tsl], dsl, xnc)

                # ---------------- phase B: scan over n ----------------
                nc.vector.memset(y_sb, 0.0)
                for n in range(N):
                    prev_h = {}
                    for s in range(NS):
                        ts0 = s * TS
                        # broadcast B/C rows to all partitions (PE selector
                        # matmul into PSUM), then downcast into bf16 SBUF
                        # strips so the u/ym multiplies hit the DVE 2x mode.
                        bB = scanp.tile([128, TS], BF16, tag="bB")
                        cB = scanp.tile([128, TS], BF16, tag="cB")
                        sel_b = id_sb[0:64, n : n + 1].to_broadcast([64, 128])
                        sel_c = id_sb[64:128, n : n + 1].to_broadcast([64, 128])
                        PS = min(1024, TS)
                        for j in range(TS // PS):
                            sl = slice(ts0 + j * PS, ts0 + (j + 1) * PS)
                            jl = slice(j * PS, (j + 1) * PS)
                            ps_b = psum.tile([128, PS], F32, tag="ps2a")
                            ps_c = psum.tile([128, PS], F32, tag="ps2b")
                            for k in range(PS // 512):
                                kj = slice(k * 512, (k + 1) * 512)
                                ksl = slice(sl.start + k * 512, sl.start + (k + 1) * 512)
                                nc.tensor.matmul(
                                    ps_b[:, kj], sel_b, bc_sb[0:64, ksl],
                                    start=True, stop=True,
                                )
                                nc.tensor.matmul(
                                    ps_c[:, kj], sel_c, bc_sb[64:128, ksl],
                                    start=True, stop=True,
                                )
                            nc.scalar.copy(bB[:, jl], ps_b)
                            nc.scalar.copy(cB[:, jl], ps_c)
                        for c in range(NCH):
                            ssl = slice(ts0, ts0 + TS)
                            dA = scanp.tile([128, TS], BF16, tag="dA")
                            nc.scalar.activation(
                                dA, delta_sb[:, c, ssl], AF.Exp,
                                scale=a_sb[:, c, n : n + 1],
                            )
                            u = scanp.tile([128, TS], BF16, tag="u")
                            nc.vector.tensor_mul(u, dxn_sb[:, c, ssl], bB)
                            h = hpool.tile([128, TS], BF16, tag="h")
                            if s == 0:
                                init = (
                                    0.0
                                    if half == 0
                                    else hcarry[:, c * N + n : c * N + n + 1]
                                )
                            else:
                                init = prev_h[c][:, TS - 1 : TS]
                            nc.vector.tensor_tensor_scan(
                                h, dA, u, init, op0=ALU.mult, op1=ALU.add
                            )
                            prev_h[c] = h
                            if half == 0 and s == NS - 1:
                                nc.gpsimd.tensor_copy(
                                    hcarry[:, c * N + n : c * N + n + 1],
                                    h[:, TS - 1 : TS],
                                )
                            ym = scanp.tile([128, TS], BF16, tag="u")
                            nc.vector.tensor_mul(ym, h, cB)
                            ysl = y_sb[:, c, ssl]
                            nc.gpsimd.tensor_add(ysl, ysl, ym)

                # ---------------- phase C: out_proj partial ----------------
                for itb in range(NTB):
                    t0 = t0h + itb * TB
                    tsl = slice(itb * TB, (itb + 1) * TB)
                    zs = []
                    for c in range(NCH):
                        xr = cpool.tile([128, TB], F32R, tag="xr")
                        nc.sync.dma_start(xr, xT[c * 128 : (c + 1) * 128, t0 : t0 + TB])
                        xz = cpool.tile([128, TB], F32, tag="xz")
                        nc.vector.tensor_scalar_mul(xz, xr.bitcast(F32), dp_sb[:, c : c + 1])
                        z = zpool.tile([128, TB], F32R, tag=f"z{c}")
                        nc.vector.tensor_add(z, y_sb[:, c, tsl], xz)
                        zs.append(z)
                    for o in range(NCHALL):
                        ps_o = psum.tile(
                            [128, TB], F32, tag="ps1a" if o % 2 == 0 else "ps1b"
                        )
                        for c in range(NCH):
                            wo = wpool.tile([128, 128], F32R, tag="wo")
                            nc.sync.dma_start(
                                wo,
                                woT[c * 128 : (c + 1) * 128, o * 128 : (o + 1) * 128],
                            )
                            nc.tensor.matmul(
                                ps_o, wo, zs[c],
                                start=(c == 0), stop=(c == NCH - 1),
                            )
                        ostg = cpool.tile([128, TB], F32, tag="xz")
                        nc.scalar.copy(ostg, ps_o)
                        nc.sync.dma_start(
                            out_part[o * 128 : (o + 1) * 128, t0 : t0 + TB], ostg
                        )
    return nc


def _get_program(L_):
    if L_ not in _PROGRAM_CACHE:
        import concourse.bacc as bacc

        nc = bacc.Bacc(None, target_bir_lowering=False)
        _build(nc, L_)
        nc.compile()
        _PROGRAM_CACHE[L_] = nc
    return _PROGRAM_CACHE[L_]


def _cols(v):
    """[DQ] -> [128, NCH] per-partition column layout (chunk-major)."""
    return np.ascontiguousarray(v.reshape(NCH, 128).T).astype(np.float32)


HW_EXEC_NS = None
_NEFF_CACHE = {}


def _profiled_run(nc, in_maps):
    """Run via PJRT with the terminal-side NRT profiler capturing NTFFs,
    then extract device exec time with neuron-profile. Falls back to an
    unprofiled run on any failure."""
    global HW_EXEC_NS
    import glob as globmod
    import json
    import subprocess
    import tempfile
    import types
    from dataclasses import dataclass

    from concourse import bass2jax

    try:
        sys.path.insert(0, "/root/.axon_site")
        from trn_agent_boot.trn_boot import _ntff_profile_via_ctypes

        hook = _ntff_profile_via_ctypes("/opt/axon/libaxon_pjrt.so")
        assert hook is not None
        neff_dir = tempfile.mkdtemp(prefix="ssmprof_")
        with hook(neff_dir, [0]):
            results = bass2jax.run_bass_via_pjrt(nc, in_maps, n_cores=8)
        ntffs = sorted(globmod.glob(os.path.join(neff_dir, "*.ntff")))
        if not ntffs:
            print("profiling: no NTFF captured")
        else:
            # the capture ships the terminal-side NEFF next to the NTFF
            neffs = sorted(globmod.glob(os.path.join(neff_dir, "*.neff")))
            neff = neffs[0]
            out_json = os.path.join(neff_dir, "prof.json")
            subprocess.run(
                ["neuron-profile", "view", "-n", neff, "-s", ntffs[0],
                 "--output-format=json", "--output-file", out_json,
                 "--ignore-nc-buf-usage"],
                check=True, env=dict(os.environ, NEURON_PROFILE_DBG_OUTPUT="2"),
                capture_output=True, text=True,
            )
            with open(out_json) as f:
                prof = json.load(f)
            insts = prof.get("instruction", [])
            if insts:
                t0 = min(i["timestamp"] for i in insts)
                t1 = max(i["timestamp"] + i.get("duration", 0) for i in insts)
                HW_EXEC_NS = int(t1 - t0)
            else:
                summ = prof.get("summary", {})
                HW_EXEC_NS = summ.get("total_time_ns")
            print(f"profiled exec: {HW_EXEC_NS} ns; json: {out_json}")

        @dataclass
        class _R:
            results: list
            exec_time_ns: object
            instructions_and_trace: object = None

        return _R(results=results, exec_time_ns=HW_EXEC_NS)
    except Exception as e:
        print(f"profiling failed ({type(e).__name__}: {e}); plain run")
        from concourse.bass_utils import run_bass_kernel_spmd as _run

        return _run(nc, in_maps, core_ids=list(range(8)), trace=False)


def kernel(
    x, norm_w, norm_b, x_proj_w, dt_proj_w, dt_proj_b, A_log, D_param, out_proj_w
):
    global LAST_RESULTS
    L_ = x.shape[1]
    nc = _get_program(L_)

    # host-side weight prep (small tensors only)
    wxF = (norm_w[:, None] * x_proj_w.T).astype(np.float32)  # [D, 256]
    g0 = (norm_w @ x_proj_w.T).astype(np.float32)  # [256]
    c0 = (norm_b @ x_proj_w.T).astype(np.float32)
    g0c = np.ascontiguousarray(g0.reshape(2, 128).T).astype(np.float32)
    c0c = np.ascontiguousarray(c0.reshape(2, 128).T).astype(np.float32)
    A = (-np.exp(A_log.astype(np.float64))).astype(np.float32)  # [D, N]
    dtwT_full = np.ascontiguousarray(dt_proj_w.T).astype(np.float32)  # [R, D]
    woT_full = np.ascontiguousarray(out_proj_w.T).astype(np.float32)  # [D, D]

    in_maps = []
    for core in range(8):
        b, q = core // 4, core % 4
        sl = slice(DQ * q, DQ * (q + 1))
        own = np.arange(DQ * q, DQ * (q + 1))
        perm = np.concatenate([own, np.delete(np.arange(D), own)])
        acols = np.ascontiguousarray(
            A[sl].reshape(NCH, 128, N).transpose(1, 0, 2)
        ).astype(np.float32)
        in_maps.append(
            {
                "xT": np.ascontiguousarray(x[b].T[perm]).astype(np.float32),
                "wxF": np.ascontiguousarray(wxF[perm]),
                "g0c": g0c,
                "c0c": c0c,
                "dtwT": np.ascontiguousarray(dtwT_full[:, sl]),
                "dtbc": _cols(dt_proj_b[sl]),
                "acols": acols,
                "wbc": _cols(norm_w[sl]),
                "bbc": _cols(norm_b[sl]),
                "dpc": _cols(D_param[sl]),
                "woT": np.ascontiguousarray(woT_full[sl]),
                "idc": np.tile(np.eye(N, dtype=np.float32), (2, 1)),
                "onesc": np.ones((128, 128), np.float32),
            }
        )

    trace = bool(int(os.environ.get("SSM_TRACE", "0")))
    if trace:
        results = _profiled_run(nc, in_maps)
        LAST_RESULTS = results
    else:
        LAST_RESULTS = run_bass_kernel_spmd(
            nc, in_maps, core_ids=list(range(8)), trace=False
        )
    parts = [r["out_part"] for r in LAST_RESULTS.results]
    out = np.stack(
        [
            (parts[0] + parts[1] + parts[2] + parts[3]).T,
            (parts[4] + parts[5] + parts[6] + parts[7]).T,
        ]
    ).astype(np.float32)
    return out



# revision 5
# speedup vs baseline: 1.1935x; 1.1935x over previous
"""Trainium2 Bass kernel for a selective-SSM block (LN -> x_proj ->
softplus(dt_proj) -> diagonal SSM scan over L -> out_proj).

Sharding: 8 cores = 2 batches x 4 D-quarters. Each core runs the scan for its
512 channels over the full sequence (channels on partitions, time on the free
dim, one tensor_tensor_scan per SSM state index n). out_proj partials (each
core contracts over its own 512 channels) are summed on the host.

Layout trick: the host passes x[b].T with rows permuted so each core's own
D-quarter occupies chunk rows 0..511; all d-contractions (LN stats, x_proj)
are permutation-invariant because the matching weight rows are permuted too.
"""

import os
import sys

import numpy as np

try:
    import concourse.bass as bass
except ImportError:
    sys.path.insert(0, "/opt/trn_rl_repo")
    import concourse.bass as bass

import concourse.tile as tile
from concourse import mybir
from concourse.bass_utils import run_bass_kernel_spmd

F32 = mybir.dt.float32
F32R = mybir.dt.float32r
BF16 = mybir.dt.bfloat16
AF = mybir.ActivationFunctionType
ALU = mybir.AluOpType

B, L, D, N, R = 2, 4096, 2048, 64, 128
P_PROJ = 2 * N + R  # 256
DQ = D // 4  # channels per core: 512
NCH = DQ // 128  # own d-chunks: 4
NCHALL = D // 128  # all d-chunks: 16
EPS = 1e-5

LAST_RESULTS = None  # BassKernelResults of the most recent run (for test.py)

_PROGRAM_CACHE = {}


def _r(ap):
    return ap.bitcast(F32R)


def _build(nc, L_):
    LH = L_ // 2  # half length
    TB = min(512, LH)  # phase A/C time block
    NTB = LH // TB
    TS = LH  # scan time block (whole half in one scan)

    xT = nc.dram_tensor("xT", [D, L_], F32R, kind="ExternalInput")
    wxF = nc.dram_tensor("wxF", [D, P_PROJ], F32R, kind="ExternalInput")
    g0c = nc.dram_tensor("g0c", [128, 2], F32, kind="ExternalInput")
    c0c = nc.dram_tensor("c0c", [128, 2], F32, kind="ExternalInput")
    dtwT = nc.dram_tensor("dtwT", [R, DQ], F32R, kind="ExternalInput")
    dtbc = nc.dram_tensor("dtbc", [128, NCH], F32, kind="ExternalInput")
    acols = nc.dram_tensor("acols", [128, NCH, N], F32, kind="ExternalInput")
    wbc = nc.dram_tensor("wbc", [128, NCH], F32, kind="ExternalInput")
    bbc = nc.dram_tensor("bbc", [128, NCH], F32, kind="ExternalInput")
    dpc = nc.dram_tensor("dpc", [128, NCH], F32, kind="ExternalInput")
    woT = nc.dram_tensor("woT", [DQ, D], F32R, kind="ExternalInput")
    idc = nc.dram_tensor("idc", [128, N], F32R, kind="ExternalInput")
    onesc = nc.dram_tensor("onesc", [128, 128], F32R, kind="ExternalInput")
    out_part = nc.dram_tensor("out_part", [D, L_], F32, kind="ExternalOutput")

    with tile.TileContext(nc) as tc:
        with (
            tc.tile_pool(name="single", bufs=1) as single,
            tc.tile_pool(name="persist", bufs=1) as persist,
            tc.tile_pool(name="xin", bufs=2) as xin,
            tc.tile_pool(name="xown", bufs=5) as xown,
            tc.tile_pool(name="wrk", bufs=2) as wrk,
            tc.tile_pool(name="stats", bufs=1) as stats,
            tc.tile_pool(name="stats2", bufs=1) as stats2,
            tc.tile_pool(name="wpool", bufs=3) as wpool,
            tc.tile_pool(name="dapool", bufs=2) as dapool,
            tc.tile_pool(name="upool", bufs=2) as upool,
            tc.tile_pool(name="ympool", bufs=3) as ympool,
            tc.tile_pool(name="hpool", bufs=2) as hpool,
            tc.tile_pool(name="bcpool", bufs=2) as bcpool,
            tc.tile_pool(name="zpool", bufs=1) as zpool,
            tc.tile_pool(name="cpool", bufs=2) as cpool,
            tc.tile_pool(name="psum", bufs=1, space=bass.MemorySpace.PSUM) as psum,
        ):
            # --- constants ---
            ones128 = single.tile([128, 128], F32R)
            nc.sync.dma_start(ones128, onesc[:, :])
            id_sb = single.tile([128, N], F32R)
            nc.sync.dma_start(id_sb, idc[:, :])
            eps_sb = single.tile([128, 1], F32)
            nc.vector.memset(eps_sb, EPS)
            g0_sb = single.tile([128, 2], F32)
            nc.sync.dma_start(g0_sb, g0c[:, :])
            c0_sb = single.tile([128, 2], F32)
            nc.sync.dma_start(c0_sb, c0c[:, :])
            dtb_sb = single.tile([128, NCH], F32)
            nc.sync.dma_start(dtb_sb, dtbc[:, :])
            a_sb = single.tile([128, NCH, N], F32)
            nc.sync.dma_start(a_sb, acols[:, :, :])
            w_sb = single.tile([128, NCH], F32)
            nc.sync.dma_start(w_sb, wbc[:, :])
            b_sb = single.tile([128, NCH], F32)
            nc.sync.dma_start(b_sb, bbc[:, :])
            dp_sb = single.tile([128, NCH], F32)
            nc.sync.dma_start(dp_sb, dpc[:, :])
            dtw_sb = single.tile([128, NCH, 128], F32R)
            for c in range(NCH):
                nc.sync.dma_start(dtw_sb[:, c, :], dtwT[:, c * 128 : (c + 1) * 128])

            # persistent per-half buffers
            delta_sb = persist.tile([128, NCH, LH], F32, tag="delta")
            dxn_sb = persist.tile([128, NCH, LH], BF16, tag="dxn")
            bc_sb = persist.tile([128, LH], F32R, tag="bc")  # rows 0-63 B, 64-127 C
            y_sb = persist.tile([128, NCH, LH], F32, tag="y")
            hcarry = persist.tile([128, NCH * N], F32, tag="hcarry")

            for half in range(2):
                t0h = half * LH
                # ---------------- phase A ----------------
                for itb in range(NTB):
                    t0 = t0h + itb * TB
                    tsl = slice(itb * TB, (itb + 1) * TB)
                    ps_sx = psum.tile([128, TB], F32, tag="ps1a")
                    ps_sxx = psum.tile([128, TB], F32, tag="ps1b")
                    ps_g0 = psum.tile([128, TB], F32, tag="ps2a")
                    ps_g1 = psum.tile([128, TB], F32, tag="ps2b")
                    own_tiles = {}
                    for c in range(NCHALL):
                        if c < NCH:  # own-quarter chunk (host row-permutation)
                            xc = xown.tile([128, TB], F32R, tag="xown")
                            own_tiles[c] = xc
                        else:
                            xc = xin.tile([128, TB], F32R, tag="xin")
                        nc.sync.dma_start(xc, xT[c * 128 : (c + 1) * 128, t0 : t0 + TB])
                        x2 = wrk.tile([128, TB], F32R, tag="wa")
                        nc.scalar.square(x2, xc.bitcast(F32))
                        st = c == 0
                        sp = c == NCHALL - 1
                        nc.tensor.matmul(ps_sx, ones128, xc, start=st, stop=sp)
                        nc.tensor.matmul(ps_sxx, ones128, x2, start=st, stop=sp)
                        wx = wpool.tile([128, P_PROJ], F32R, tag="wx")
                        nc.sync.dma_start(wx, wxF[c * 128 : (c + 1) * 128, :])
                        nc.tensor.matmul(ps_g0, wx[:, 0:128], xc, start=st, stop=sp)
                        nc.tensor.matmul(ps_g1, wx[:, 128:256], xc, start=st, stop=sp)

                    # stats: mean, rstd (all partition rows carry the same value)
                    mean_b = stats.tile([128, TB], F32, tag="mean")
                    nc.scalar.mul(mean_b, ps_sx, 1.0 / D)
                    msq = stats.tile([128, TB], F32, tag="sA")
                    nc.scalar.mul(msq, ps_sxx, 1.0 / D)
                    m2 = stats.tile([128, TB], F32, tag="sB")
                    nc.vector.tensor_mul(m2, mean_b, mean_b)
                    nc.vector.tensor_sub(msq, msq, m2)  # msq <- var
                    # rstd = exp(-0.5*ln(var+eps)): stays inside the exp/ln ACT set
                    nc.scalar.activation(m2, msq, AF.Ln, bias=eps_sb[:, 0:1])
                    rstd_b = stats2.tile([128, TB], F32, tag="rstd")
                    nc.scalar.activation(rstd_b, m2, AF.Exp, scale=-0.5)
                    mr_b = stats2.tile([128, TB], F32, tag="mr")
                    nc.vector.tensor_mul(mr_b, mean_b, rstd_b)

                    # proj = rstd*G - (mr*g0 - c0)   (LN folded into x_proj)
                    dr_sb = wrk.tile([128, TB], F32R, tag="drt")
                    for ph, ps_g in enumerate([ps_g0, ps_g1]):
                        s1 = wrk.tile([128, TB], F32, tag="wb")
                        nc.vector.tensor_mul(s1, ps_g, rstd_b)
                        s2 = wrk.tile([128, TB], F32, tag="wc")
                        nc.vector.tensor_scalar(
                            s2,
                            mr_b,
                            g0_sb[:, ph : ph + 1],
                            c0_sb[:, ph : ph + 1],
                            op0=ALU.mult,
                            op1=ALU.subtract,
                        )
                        tgt = dr_sb if ph == 0 else bc_sb[:, tsl]
                        nc.vector.tensor_sub(tgt, s1, s2)

                    # dt_proj + softplus -> delta; xn + delta*xn for own chunks
                    for c in range(NCH):
                        ps_dt = psum.tile([128, TB], F32, tag="ps1c")
                        nc.tensor.matmul(
                            ps_dt, dtw_sb[:, c, :], dr_sb, start=True, stop=True
                        )
                        # softplus(z) = relu(z) + ln(1 + exp(-|z|)); all four
                        # ACT funcs live in the natural_log_exp table set
                        dsl = delta_sb[:, c, tsl]
                        t_abs = wrk.tile([128, TB], F32, tag="wa")
                        nc.scalar.activation(
                            t_abs, ps_dt, AF.Abs, bias=dtb_sb[:, c : c + 1]
                        )
                        nc.scalar.activation(t_abs, t_abs, AF.Exp, scale=-1.0)
                        nc.scalar.activation(t_abs, t_abs, AF.Ln, bias=1.0)
                        t_r = wrk.tile([128, TB], F32, tag="wb")
                        nc.scalar.activation(
                            t_r, ps_dt, AF.Relu, bias=dtb_sb[:, c : c + 1]
                        )
                        nc.vector.tensor_add(dsl, t_abs, t_r)
                        xc = own_tiles[c]
                        t1 = wrk.tile([128, TB], F32, tag="wa")
                        nc.vector.tensor_mul(t1, xc.bitcast(F32), rstd_b)
                        t2 = wrk.tile([128, TB], F32, tag="wb")
                        nc.vector.tensor_sub(t2, t1, mr_b)
                        xnc = wrk.tile([128, TB], F32, tag="wc")
                        nc.scalar.activation(
                            xnc,
                            t2,
                            AF.Identity,
                            bias=b_sb[:, c : c + 1],
                            scale=w_sb[:, c : c + 1],
                        )
                        nc.vector.tensor_mul(dxn_sb[:, c, tsl], dsl, xnc)

                # ---------------- phase B: scan over n ----------------
                nc.vector.memset(y_sb, 0.0)
                for n in range(N):
                    # broadcast B/C rows to all partitions (PE selector
                    # matmul into PSUM), then downcast into bf16 SBUF
                    # strips so the u/ym multiplies hit the DVE 2x mode.
                    bB = bcpool.tile([128, TS], BF16, tag="bB")
                    cB = bcpool.tile([128, TS], BF16, tag="cB")
                    sel_b = id_sb[0:64, n : n + 1].to_broadcast([64, 128])
                    sel_c = id_sb[64:128, n : n + 1].to_broadcast([64, 128])
                    PS = min(1024, TS)
                    for j in range(TS // PS):
                        jl = slice(j * PS, (j + 1) * PS)
                        ps_b = psum.tile([128, PS], F32, tag="ps2a")
                        ps_c = psum.tile([128, PS], F32, tag="ps2b")
                        for k in range(PS // 512):
                            kj = slice(k * 512, (k + 1) * 512)
                            ksl = slice(jl.start + k * 512, jl.start + (k + 1) * 512)
                            nc.tensor.matmul(
                                ps_b[:, kj], sel_b, bc_sb[0:64, ksl],
                                start=True, stop=True,
                            )
                            nc.tensor.matmul(
                                ps_c[:, kj], sel_c, bc_sb[64:128, ksl],
                                start=True, stop=True,
                            )
                        nc.scalar.copy(bB[:, jl], ps_b)
                        nc.scalar.copy(cB[:, jl], ps_c)
                    for c in range(NCH):
                        dA = dapool.tile([128, TS], BF16, tag="dA")
                        nc.scalar.activation(
                            dA, delta_sb[:, c, :], AF.Exp,
                            scale=a_sb[:, c, n : n + 1],
                        )
                        u = upool.tile([128, TS], BF16, tag="u")
                        nc.vector.tensor_mul(u, dxn_sb[:, c, :], bB)
                        h = hpool.tile([128, TS], BF16, tag="h")
                        init = (
                            0.0
                            if half == 0
                            else hcarry[:, c * N + n : c * N + n + 1]
                        )
                        nc.vector.tensor_tensor_scan(
                            h, dA, u, init, op0=ALU.mult, op1=ALU.add
                        )
                        if half == 0:
                            nc.gpsimd.tensor_copy(
                                hcarry[:, c * N + n : c * N + n + 1],
                                h[:, TS - 1 : TS],
                            )
                        ym = ympool.tile([128, TS], BF16, tag="ym")
                        nc.vector.tensor_mul(ym, h, cB)
                        ysl = y_sb[:, c, :]
                        nc.gpsimd.tensor_add(ysl, ysl, ym)

                # ---------------- phase C: out_proj partial ----------------
                for itb in range(NTB):
                    t0 = t0h + itb * TB
                    tsl = slice(itb * TB, (itb + 1) * TB)
                    zs = []
                    for c in range(NCH):
                        xr = cpool.tile([128, TB], F32R, tag="xr")
                        nc.sync.dma_start(xr, xT[c * 128 : (c + 1) * 128, t0 : t0 + TB])
                        xz = cpool.tile([128, TB], F32, tag="xz")
                        nc.vector.tensor_scalar_mul(xz, xr.bitcast(F32), dp_sb[:, c : c + 1])
                        z = zpool.tile([128, TB], F32R, tag=f"z{c}")
                        nc.vector.tensor_add(z, y_sb[:, c, tsl], xz)
                        zs.append(z)
                    for o in range(NCHALL):
                        ps_o = psum.tile(
                            [128, TB], F32, tag="ps1a" if o % 2 == 0 else "ps1b"
                        )
                        for c in range(NCH):
                            wo = wpool.tile([128, 128], F32R, tag="wo")
                            nc.sync.dma_start(
                                wo,
                                woT[c * 128 : (c + 1) * 128, o * 128 : (o + 1) * 128],
                            )
                            nc.tensor.matmul(
                                ps_o, wo, zs[c],
                                start=(c == 0), stop=(c == NCH - 1),
                            )
                        ostg = cpool.tile([128, TB], F32, tag="xz")
                        nc.scalar.copy(ostg, ps_o)
                        nc.sync.dma_start(
                            out_part[o * 128 : (o + 1) * 128, t0 : t0 + TB], ostg
                        )
    return nc


def _get_program(L_):
    if L_ not in _PROGRAM_CACHE:
        import concourse.bacc as bacc

        nc = bacc.Bacc(None, target_bir_lowering=False)
        _build(nc, L_)
        nc.compile()
        _PROGRAM_CACHE[L_] = nc
    return _PROGRAM_CACHE[L_]


def _cols(v):
    """[DQ] -> [128, NCH] per-partition column layout (chunk-major)."""
    return np.ascontiguousarray(v.reshape(NCH, 128).T).astype(np.float32)


HW_EXEC_NS = None
_NEFF_CACHE = {}


def _profiled_run(nc, in_maps):
    """Run via PJRT with the terminal-side NRT profiler capturing NTFFs,
    then extract device exec time with neuron-profile. Falls back to an
    unprofiled run on any failure."""
    global HW_EXEC_NS
    import glob as globmod
    import json
    import subprocess
    import tempfile
    import types
    from dataclasses import dataclass

    from concourse import bass2jax

    try:
        sys.path.insert(0, "/root/.axon_site")
        from trn_agent_boot.trn_boot import _ntff_profile_via_ctypes

        hook = _ntff_profile_via_ctypes("/opt/axon/libaxon_pjrt.so")
        assert hook is not None
        neff_dir = tempfile.mkdtemp(prefix="ssmprof_")
        with hook(neff_dir, [0]):
            results = bass2jax.run_bass_via_pjrt(nc, in_maps, n_cores=8)
        ntffs = sorted(globmod.glob(os.path.join(neff_dir, "*.ntff")))
        if not ntffs:
            print("profiling: no NTFF captured")
        else:
            # the capture ships the terminal-side NEFF next to the NTFF
            neffs = sorted(globmod.glob(os.path.join(neff_dir, "*.neff")))
            neff = neffs[0]
            out_json = os.path.join(neff_dir, "prof.json")
            subprocess.run(
                ["neuron-profile", "view", "-n", neff, "-s", ntffs[0],
                 "--output-format=json", "--output-file", out_json,
                 "--ignore-nc-buf-usage"],
                check=True, env=dict(os.environ, NEURON_PROFILE_DBG_OUTPUT="2"),
                capture_output=True, text=True,
            )
            with open(out_json) as f:
                prof = json.load(f)
            insts = prof.get("instruction", [])
            if insts:
                t0 = min(i["timestamp"] for i in insts)
                t1 = max(i["timestamp"] + i.get("duration", 0) for i in insts)
                HW_EXEC_NS = int(t1 - t0)
            else:
                summ = prof.get("summary", {})
                HW_EXEC_NS = summ.get("total_time_ns")
            print(f"profiled exec: {HW_EXEC_NS} ns; json: {out_json}")

        @dataclass
        class _R:
            results: list
            exec_time_ns: object
            instructions_and_trace: object = None

        return _R(results=results, exec_time_ns=HW_EXEC_NS)
    except Exception as e:
        print(f"profiling failed ({type(e).__name__}: {e}); plain run")
        from concourse.bass_utils import run_bass_kernel_spmd as _run

        return _run(nc, in_maps, core_ids=list(range(8)), trace=False)


def kernel(
    x, norm_w, norm_b, x_proj_w, dt_proj_w, dt_proj_b, A_log, D_param, out_proj_w
):
    global LAST_RESULTS
    L_ = x.shape[1]
    nc = _get_program(L_)

    # host-side weight prep (small tensors only)
    wxF = (norm_w[:, None] * x_proj_w.T).astype(np.float32)  # [D, 256]
    g0 = (norm_w @ x_proj_w.T).astype(np.float32)  # [256]
    c0 = (norm_b @ x_proj_w.T).astype(np.float32)
    g0c = np.ascontiguousarray(g0.reshape(2, 128).T).astype(np.float32)
    c0c = np.ascontiguousarray(c0.reshape(2, 128).T).astype(np.float32)
    A = (-np.exp(A_log.astype(np.float64))).astype(np.float32)  # [D, N]
    dtwT_full = np.ascontiguousarray(dt_proj_w.T).astype(np.float32)  # [R, D]
    woT_full = np.ascontiguousarray(out_proj_w.T).astype(np.float32)  # [D, D]

    in_maps = []
    for core in range(8):
        b, q = core // 4, core % 4
        sl = slice(DQ * q, DQ * (q + 1))
        own = np.arange(DQ * q, DQ * (q + 1))
        perm = np.concatenate([own, np.delete(np.arange(D), own)])
        acols = np.ascontiguousarray(
            A[sl].reshape(NCH, 128, N).transpose(1, 0, 2)
        ).astype(np.float32)
        in_maps.append(
            {
                "xT": np.ascontiguousarray(x[b].T[perm]).astype(np.float32),
                "wxF": np.ascontiguousarray(wxF[perm]),
                "g0c": g0c,
                "c0c": c0c,
                "dtwT": np.ascontiguousarray(dtwT_full[:, sl]),
                "dtbc": _cols(dt_proj_b[sl]),
                "acols": acols,
                "wbc": _cols(norm_w[sl]),
                "bbc": _cols(norm_b[sl]),
                "dpc": _cols(D_param[sl]),
                "woT": np.ascontiguousarray(woT_full[sl]),
                "idc": np.tile(np.eye(N, dtype=np.float32), (2, 1)),
                "onesc": np.ones((128, 128), np.float32),
            }
        )

    trace = bool(int(os.environ.get("SSM_TRACE", "0")))
    if trace:
        results = _profiled_run(nc, in_maps)
        LAST_RESULTS = results
    else:
        LAST_RESULTS = run_bass_kernel_spmd(
            nc, in_maps, core_ids=list(range(8)), trace=False
        )
    parts = [r["out_part"] for r in LAST_RESULTS.results]
    out = np.stack(
        [
            (parts[0] + parts[1] + parts[2] + parts[3]).T,
            (parts[4] + parts[5] + parts[6] + parts[7]).T,
        ]
    ).astype(np.float32)
    return out



# revision 9
# speedup vs baseline: 1.6826x; 1.4098x over previous
"""Trainium2 Bass kernel for a selective-SSM block (LN -> x_proj ->
softplus(dt_proj) -> diagonal SSM scan over L -> out_proj).

Sharding: 8 cores = 2 batches x 4 D-quarters. Each core runs the scan for its
512 channels over the full sequence (channels on partitions, time on the free
dim, one tensor_tensor_scan per SSM state index n). out_proj partials (each
core contracts over its own 512 channels) are summed on the host.

Engine assignment in the scan phase (phase B) avoids the DVE/GpSimd shared
SBUF port entirely: Vector does u-mul/scan/ym-mul, Scalar does the dA
exponentials and state-carry copies, the B/C row broadcasts ride the DMA
(AXI) ports via a DRAM bounce + stride-0-partition reads, and the sum over
the 64 SSM states accumulates on the Tensor engine (identity matmul into
PSUM). GpSimd issues nothing in phase B.

Layout trick: the host passes x[b].T with rows permuted so each core's own
D-quarter occupies chunk rows 0..511; all d-contractions (LN stats, x_proj)
are permutation-invariant because the matching weight rows are permuted too.
"""

import os
import sys

import numpy as np

try:
    import concourse.bass as bass
except ImportError:
    sys.path.insert(0, "/opt/trn_rl_repo")
    import concourse.bass as bass

import concourse.tile as tile
from concourse import mybir
from concourse.bass_utils import run_bass_kernel_spmd

F32 = mybir.dt.float32
F32R = mybir.dt.float32r
BF16 = mybir.dt.bfloat16
AF = mybir.ActivationFunctionType
ALU = mybir.AluOpType

B, L, D, N, R = 2, 4096, 2048, 64, 128
P_PROJ = 2 * N + R  # 256
DQ = D // 4  # channels per core: 512
NCH = DQ // 128  # own d-chunks: 4
NCHALL = D // 128  # all d-chunks: 16
EPS = 1e-5

LAST_RESULTS = None  # BassKernelResults of the most recent run (for test.py)

_PROGRAM_CACHE = {}


def _r(ap):
    return ap.bitcast(F32R)


def _scan_block(nc, half, n, c, ypc, bB, cB, delta_sb, dxn_sb, a_sb, hcarry,
                dapool, upool, ympool, hpool, id_sb, LH, NQ):
    """One (state n, chunk c) unit of phase B: dA exp (Scalar), u-mul / scan /
    ym-mul (Vector), y-accumulate (Tensor identity MM into PSUM)."""
    dA = dapool.tile([128, LH], BF16, tag="dA")
    nc.scalar.activation(
        dA, delta_sb[:, c, :], AF.Exp, scale=a_sb[:, c, n : n + 1]
    )
    u = upool.tile([128, LH], BF16, tag="u")
    nc.vector.tensor_mul(u, dxn_sb[:, c, :], bB)
    h = hpool.tile([128, LH], BF16, tag="h")
    init = 0.0 if half == 0 else hcarry[:, c * N + n : c * N + n + 1]
    nc.vector.tensor_tensor_scan(h, dA, u, init, op0=ALU.mult, op1=ALU.add)
    if half == 0:
        nc.scalar.copy(
            hcarry[:, c * N + n : c * N + n + 1], h[:, LH - 1 : LH]
        )
    ym = ympool.tile([128, LH], BF16, tag="ym")
    nc.vector.tensor_mul(ym, h, cB)
    for q in range(NQ):
        ql = slice(q * 512, (q + 1) * 512)
        nc.tensor.matmul(
            ypc[:, ql], id_sb, ym[:, ql], start=(n == 0), stop=(n == N - 1)
        )


def _build(nc, L_):
    LH = L_ // 2  # half length
    TB = min(512, LH)  # phase A/C time block
    NTB = LH // TB
    NQ = LH // 512  # 512-col sub-blocks for PE y-accumulate

    xT = nc.dram_tensor("xT", [D, L_], F32R, kind="ExternalInput")
    wxF = nc.dram_tensor("wxF", [D, P_PROJ], F32R, kind="ExternalInput")
    g0c = nc.dram_tensor("g0c", [128, 2], F32, kind="ExternalInput")
    c0c = nc.dram_tensor("c0c", [128, 2], F32, kind="ExternalInput")
    dtwT = nc.dram_tensor("dtwT", [R, DQ], F32R, kind="ExternalInput")
    dtbc = nc.dram_tensor("dtbc", [128, NCH], F32, kind="ExternalInput")
    acols = nc.dram_tensor("acols", [128, NCH, N], F32, kind="ExternalInput")
    wbc = nc.dram_tensor("wbc", [128, NCH], F32, kind="ExternalInput")
    bbc = nc.dram_tensor("bbc", [128, NCH], F32, kind="ExternalInput")
    dpc = nc.dram_tensor("dpc", [128, NCH], F32, kind="ExternalInput")
    woT = nc.dram_tensor("woT", [DQ, D], F32R, kind="ExternalInput")
    idbf = nc.dram_tensor("idbf", [128, 128], BF16, kind="ExternalInput")
    onesc = nc.dram_tensor("onesc", [128, 128], F32R, kind="ExternalInput")
    bcd = nc.dram_tensor("bcd", [128, LH], BF16, kind="Internal")
    out_part = nc.dram_tensor("out_part", [D, L_], F32, kind="ExternalOutput")

    with tile.TileContext(nc) as tc:
        with (
            tc.tile_pool(name="single", bufs=1) as single,
            tc.tile_pool(name="persist", bufs=1) as persist,
            tc.tile_pool(name="xin", bufs=2) as xin,
            tc.tile_pool(name="xown", bufs=5) as xown,
            tc.tile_pool(name="wrk", bufs=2) as wrk,
            tc.tile_pool(name="stats", bufs=1) as stats,
            tc.tile_pool(name="stats2", bufs=1) as stats2,
            tc.tile_pool(name="wpool", bufs=3) as wpool,
            tc.tile_pool(name="dapool", bufs=2) as dapool,
            tc.tile_pool(name="upool", bufs=2) as upool,
            tc.tile_pool(name="ympool", bufs=3) as ympool,
            tc.tile_pool(name="hpool", bufs=2) as hpool,
            tc.tile_pool(name="bcpool", bufs=2) as bcpool,
            tc.tile_pool(name="zpool", bufs=1) as zpool,
            tc.tile_pool(name="cpool", bufs=2) as cpool,
            tc.tile_pool(name="psum", bufs=1, space=bass.MemorySpace.PSUM) as psum,
        ):
            # --- constants ---
            ones128 = single.tile([128, 128], F32R)
            nc.sync.dma_start(ones128, onesc[:, :])
            id_sb = single.tile([128, 128], BF16)
            nc.sync.dma_start(id_sb, idbf[:, :])
            eps_sb = single.tile([128, 1], F32)
            nc.vector.memset(eps_sb, EPS)
            g0_sb = single.tile([128, 2], F32)
            nc.sync.dma_start(g0_sb, g0c[:, :])
            c0_sb = single.tile([128, 2], F32)
            nc.sync.dma_start(c0_sb, c0c[:, :])
            dtb_sb = single.tile([128, NCH], F32)
            nc.sync.dma_start(dtb_sb, dtbc[:, :])
            a_sb = single.tile([128, NCH, N], F32)
            nc.sync.dma_start(a_sb, acols[:, :, :])
            w_sb = single.tile([128, NCH], F32)
            nc.sync.dma_start(w_sb, wbc[:, :])
            b_sb = single.tile([128, NCH], F32)
            nc.sync.dma_start(b_sb, bbc[:, :])
            dp_sb = single.tile([128, NCH], F32)
            nc.sync.dma_start(dp_sb, dpc[:, :])
            dtw_sb = single.tile([128, NCH, 128], F32R)
            for c in range(NCH):
                nc.sync.dma_start(dtw_sb[:, c, :], dtwT[:, c * 128 : (c + 1) * 128])

            # persistent per-half buffers
            delta_sb = persist.tile([128, NCH, LH], F32, tag="delta")
            dxn_sb = persist.tile([128, NCH, LH], BF16, tag="dxn")
            bc_sb = persist.tile([128, LH], BF16, tag="bc")  # rows 0-63 B, 64-127 C
            y_sb = persist.tile([128, NCH, LH], F32, tag="y")
            hcarry = persist.tile([128, NCH * N], F32, tag="hcarry")

            for half in range(2):
                t0h = half * LH
                # ---------------- phase A ----------------
                for itb in range(NTB):
                    t0 = t0h + itb * TB
                    tsl = slice(itb * TB, (itb + 1) * TB)
                    psA = psum.tile([128, 4, TB], F32, tag="yps0")
                    psB = psum.tile([128, 4, TB], F32, tag="yps1")
                    ps_sx = psA[:, 0, :]
                    ps_sxx = psA[:, 1, :]
                    ps_g0 = psA[:, 2, :]
                    ps_g1 = psA[:, 3, :]
                    own_tiles = {}
                    for c in range(NCHALL):
                        if c < NCH:  # own-quarter chunk (host row-permutation)
                            xc = xown.tile([128, TB], F32R, tag="xown")
                            own_tiles[c] = xc
                        else:
                            xc = xin.tile([128, TB], F32R, tag="xin")
                        nc.sync.dma_start(xc, xT[c * 128 : (c + 1) * 128, t0 : t0 + TB])
                        x2 = wrk.tile([128, TB], F32R, tag="wa")
                        nc.scalar.square(x2, xc.bitcast(F32))
                        st = c == 0
                        sp = c == NCHALL - 1
                        nc.tensor.matmul(ps_sx, ones128, xc, start=st, stop=sp)
                        nc.tensor.matmul(ps_sxx, ones128, x2, start=st, stop=sp)
                        wx = wpool.tile([128, P_PROJ], F32R, tag="wx")
                        nc.sync.dma_start(wx, wxF[c * 128 : (c + 1) * 128, :])
                        nc.tensor.matmul(ps_g0, wx[:, 0:128], xc, start=st, stop=sp)
                        nc.tensor.matmul(ps_g1, wx[:, 128:256], xc, start=st, stop=sp)

                    # stats: mean, rstd (all partition rows carry the same value)
                    mean_b = stats.tile([128, TB], F32, tag="mean")
                    nc.scalar.mul(mean_b, ps_sx, 1.0 / D)
                    msq = stats.tile([128, TB], F32, tag="sA")
                    nc.scalar.mul(msq, ps_sxx, 1.0 / D)
                    m2 = stats.tile([128, TB], F32, tag="sB")
                    nc.vector.tensor_mul(m2, mean_b, mean_b)
                    nc.vector.tensor_sub(msq, msq, m2)  # msq <- var
                    # rstd = exp(-0.5*ln(var+eps)): stays inside the exp/ln ACT set
                    nc.scalar.activation(m2, msq, AF.Ln, bias=eps_sb[:, 0:1])
                    rstd_b = stats2.tile([128, TB], F32, tag="rstd")
                    nc.scalar.activation(rstd_b, m2, AF.Exp, scale=-0.5)
                    mr_b = stats2.tile([128, TB], F32, tag="mr")
                    nc.vector.tensor_mul(mr_b, mean_b, rstd_b)

                    # proj = rstd*G - (mr*g0 - c0)   (LN folded into x_proj)
                    dr_sb = wrk.tile([128, TB], F32R, tag="drt")
                    for ph, ps_g in enumerate([ps_g0, ps_g1]):
                        s1 = wrk.tile([128, TB], F32, tag="wb")
                        nc.vector.tensor_mul(s1, ps_g, rstd_b)
                        s2 = wrk.tile([128, TB], F32, tag="wc")
                        nc.vector.tensor_scalar(
                            s2,
                            mr_b,
                            g0_sb[:, ph : ph + 1],
                            c0_sb[:, ph : ph + 1],
                            op0=ALU.mult,
                            op1=ALU.subtract,
                        )
                        tgt = dr_sb if ph == 0 else bc_sb[:, tsl]
                        nc.vector.tensor_sub(tgt, s1, s2)

                    # dt_proj + softplus -> delta; xn + delta*xn for own chunks
                    for c in range(NCH):
                        ps_dt = psB[:, c, :]
                        nc.tensor.matmul(
                            ps_dt, dtw_sb[:, c, :], dr_sb, start=True, stop=True
                        )
                        # softplus(z) = relu(z) + ln(1 + exp(-|z|)); all four
                        # ACT funcs live in the natural_log_exp table set
                        dsl = delta_sb[:, c, tsl]
                        t_abs = wrk.tile([128, TB], F32, tag="wa")
                        nc.scalar.activation(
                            t_abs, ps_dt, AF.Abs, bias=dtb_sb[:, c : c + 1]
                        )
                        nc.scalar.activation(t_abs, t_abs, AF.Exp, scale=-1.0)
                        nc.scalar.activation(t_abs, t_abs, AF.Ln, bias=1.0)
                        t_r = wrk.tile([128, TB], F32, tag="wb")
                        nc.scalar.activation(
                            t_r, ps_dt, AF.Relu, bias=dtb_sb[:, c : c + 1]
                        )
                        nc.vector.tensor_add(dsl, t_abs, t_r)
                        xc = own_tiles[c]
                        t1 = wrk.tile([128, TB], F32, tag="wa")
                        nc.vector.tensor_mul(t1, xc.bitcast(F32), rstd_b)
                        t2 = wrk.tile([128, TB], F32, tag="wb")
                        nc.vector.tensor_sub(t2, t1, mr_b)
                        xnc = wrk.tile([128, TB], F32, tag="wc")
                        nc.scalar.activation(
                            xnc,
                            t2,
                            AF.Identity,
                            bias=b_sb[:, c : c + 1],
                            scale=w_sb[:, c : c + 1],
                        )
                        nc.vector.tensor_mul(dxn_sb[:, c, tsl], dsl, xnc)

                # bounce B/C rows to DRAM for the stride-0 broadcast reads
                nc.sync.dma_start(bcd[:, :], bc_sb)

                # ---------------- phase B: scan over n ----------------
                # c-pairs so two [128, LH] f32 PSUM accumulators fill PSUM
                for cpair in range(NCH // 2):
                    cs = (2 * cpair, 2 * cpair + 1)
                    yp0 = psum.tile([128, LH], F32, tag="yps0", name=f"yp0_{half}_{cpair}")
                    yp1 = psum.tile([128, LH], F32, tag="yps1", name=f"yp1_{half}_{cpair}")
                    yp = {cs[0]: yp0, cs[1]: yp1}
                    for n in range(N):
                        bB = bcpool.tile([128, LH], BF16, tag="bB")
                        nc.sync.dma_start(
                            bB, bcd[n : n + 1, :].to_broadcast([128, LH])
                        )
                        cB = bcpool.tile([128, LH], BF16, tag="cB")
                        nc.sync.dma_start(
                            cB, bcd[64 + n : 65 + n, :].to_broadcast([128, LH])
                        )
                        for c in cs:
                            _scan_block(
                                nc, half, n, c, yp[c], bB, cB, delta_sb,
                                dxn_sb, a_sb, hcarry, dapool, upool, ympool,
                                hpool, id_sb, LH, NQ,
                            )
                    for c in cs:
                        nc.scalar.copy(y_sb[:, c, :], yp[c])

                # ---------------- phase C: out_proj partial ----------------
                for itb in range(NTB):
                    t0 = t0h + itb * TB
                    tsl = slice(itb * TB, (itb + 1) * TB)
                    psC = psum.tile([128, 4, TB], F32, tag="yps0")
                    zs = []
                    for c in range(NCH):
                        xr = cpool.tile([128, TB], F32R, tag="xr")
                        nc.sync.dma_start(xr, xT[c * 128 : (c + 1) * 128, t0 : t0 + TB])
                        xz = cpool.tile([128, TB], F32, tag="xz")
                        nc.vector.tensor_scalar_mul(xz, xr.bitcast(F32), dp_sb[:, c : c + 1])
                        z = zpool.tile([128, TB], F32R, tag=f"z{c}")
                        nc.vector.tensor_add(z, y_sb[:, c, tsl], xz)
                        zs.append(z)
                    for o in range(NCHALL):
                        ps_o = psC[:, o % 4, :]
                        for c in range(NCH):
                            wo = wpool.tile([128, 128], F32R, tag="wo")
                            nc.sync.dma_start(
                                wo,
                                woT[c * 128 : (c + 1) * 128, o * 128 : (o + 1) * 128],
                            )
                            nc.tensor.matmul(
                                ps_o, wo, zs[c],
                                start=(c == 0), stop=(c == NCH - 1),
                            )
                        ostg = cpool.tile([128, TB], F32, tag="xz")
                        nc.scalar.copy(ostg, ps_o)
                        nc.sync.dma_start(
                            out_part[o * 128 : (o + 1) * 128, t0 : t0 + TB], ostg
                        )
    return nc


def _get_program(L_):
    if L_ not in _PROGRAM_CACHE:
        import concourse.bacc as bacc

        nc = bacc.Bacc(None, target_bir_lowering=False)
        _build(nc, L_)
        nc.compile()
        _PROGRAM_CACHE[L_] = nc
    return _PROGRAM_CACHE[L_]


def _cols(v):
    """[DQ] -> [128, NCH] per-partition column layout (chunk-major)."""
    return np.ascontiguousarray(v.reshape(NCH, 128).T).astype(np.float32)


HW_EXEC_NS = None
_NEFF_CACHE = {}


def _profiled_run(nc, in_maps):
    """Run via PJRT with the terminal-side NRT profiler capturing NTFFs,
    then extract device exec time with neuron-profile. Falls back to an
    unprofiled run on any failure."""
    global HW_EXEC_NS
    import glob as globmod
    import json
    import subprocess
    import tempfile
    from dataclasses import dataclass

    from concourse import bass2jax

    try:
        sys.path.insert(0, "/root/.axon_site")
        from trn_agent_boot.trn_boot import _ntff_profile_via_ctypes

        hook = _ntff_profile_via_ctypes("/opt/axon/libaxon_pjrt.so")
        assert hook is not None
        neff_dir = tempfile.mkdtemp(prefix="ssmprof_")
        with hook(neff_dir, [0]):
            results = bass2jax.run_bass_via_pjrt(nc, in_maps, n_cores=8)
        ntffs = sorted(globmod.glob(os.path.join(neff_dir, "*.ntff")))
        if not ntffs:
            print("profiling: no NTFF captured")
        else:
            # the capture ships the terminal-side NEFF next to the NTFF
            neffs = sorted(globmod.glob(os.path.join(neff_dir, "*.neff")))
            neff = neffs[0]
            out_json = os.path.join(neff_dir, "prof.json")
            subprocess.run(
                ["neuron-profile", "view", "-n", neff, "-s", ntffs[0],
                 "--output-format=json", "--output-file", out_json,
                 "--ignore-nc-buf-usage"],
                check=True, env=dict(os.environ, NEURON_PROFILE_DBG_OUTPUT="2"),
                capture_output=True, text=True,
            )
            with open(out_json) as f:
                prof = json.load(f)
            insts = prof.get("instruction", [])
            if insts:
                t0 = min(i["timestamp"] for i in insts)
                t1 = max(i["timestamp"] + i.get("duration", 0) for i in insts)
                HW_EXEC_NS = int(t1 - t0)
            else:
                summ = prof.get("summary", {})
                HW_EXEC_NS = summ.get("total_time_ns")
            print(f"profiled exec: {HW_EXEC_NS} ns; json: {out_json}")

        @dataclass
        class _R:
            results: list
            exec_time_ns: object
            instructions_and_trace: object = None

        return _R(results=results, exec_time_ns=HW_EXEC_NS)
    except Exception as e:
        print(f"profiling failed ({type(e).__name__}: {e}); plain run")
        from concourse.bass_utils import run_bass_kernel_spmd as _run

        return _run(nc, in_maps, core_ids=list(range(8)), trace=False)


def kernel(
    x, norm_w, norm_b, x_proj_w, dt_proj_w, dt_proj_b, A_log, D_param, out_proj_w
):
    global LAST_RESULTS
    import ml_dtypes

    L_ = x.shape[1]
    nc = _get_program(L_)

    # host-side weight prep (small tensors only)
    wxF = (norm_w[:, None] * x_proj_w.T).astype(np.float32)  # [D, 256]
    g0 = (norm_w @ x_proj_w.T).astype(np.float32)  # [256]
    c0 = (norm_b @ x_proj_w.T).astype(np.float32)
    g0c = np.ascontiguousarray(g0.reshape(2, 128).T).astype(np.float32)
    c0c = np.ascontiguousarray(c0.reshape(2, 128).T).astype(np.float32)
    A = (-np.exp(A_log.astype(np.float64))).astype(np.float32)  # [D, N]
    dtwT_full = np.ascontiguousarray(dt_proj_w.T).astype(np.float32)  # [R, D]
    woT_full = np.ascontiguousarray(out_proj_w.T).astype(np.float32)  # [D, D]
    idbf = np.eye(128, dtype=ml_dtypes.bfloat16)

    in_maps = []
    for core in range(8):
        b, q = core // 4, core % 4
        sl = slice(DQ * q, DQ * (q + 1))
        own = np.arange(DQ * q, DQ * (q + 1))
        perm = np.concatenate([own, np.delete(np.arange(D), own)])
        acols = np.ascontiguousarray(
            A[sl].reshape(NCH, 128, N).transpose(1, 0, 2)
        ).astype(np.float32)
        in_maps.append(
            {
                "xT": np.ascontiguousarray(x[b].T[perm]).astype(np.float32),
                "wxF": np.ascontiguousarray(wxF[perm]),
                "g0c": g0c,
                "c0c": c0c,
                "dtwT": np.ascontiguousarray(dtwT_full[:, sl]),
                "dtbc": _cols(dt_proj_b[sl]),
                "acols": acols,
                "wbc": _cols(norm_w[sl]),
                "bbc": _cols(norm_b[sl]),
                "dpc": _cols(D_param[sl]),
                "woT": np.ascontiguousarray(woT_full[sl]),
                "idbf": idbf,
                "onesc": np.ones((128, 128), np.float32),
            }
        )

    trace = bool(int(os.environ.get("SSM_TRACE", "0")))
    if trace:
        results = _profiled_run(nc, in_maps)
        LAST_RESULTS = results
    else:
        LAST_RESULTS = run_bass_kernel_spmd(
            nc, in_maps, core_ids=list(range(8)), trace=False
        )
    parts = [r["out_part"] for r in LAST_RESULTS.results]
    out = np.stack(
        [
            (parts[0] + parts[1] + parts[2] + parts[3]).T,
            (parts[4] + parts[5] + parts[6] + parts[7]).T,
        ]
    ).astype(np.float32)
    return out
